# revision 10
# baseline (speedup 1.0000x reference)
"""Bass/Trainium2 kernel for nn_EnhancedCausalTransformer.

Strategy (8 NeuronCores, SPMD, zero collectives):
  - The 128-step LSTM recurrence is the serial bottleneck (~14us/step PE
    floor streaming W_hh).  It is replicated on every core; W_hh lives in
    SBUF as bf16 [128,128] stationary tiles.
  - The pairwise mechanism MLP (4032 pairs, the only heavy parallel GEMM)
    is sharded 504 pairs/core; host concatenates the shards.
  - Everything else (state/causal encoders, causal-graph stats, heads) is
    replicated; its PE/DVE/ACT time hides under the LSTM's PE time.
  - Host does layout/dtype staging and index gathers only; all value math
    runs on device.  Outputs are taken from core 0 (mech from all cores).
"""

import os
import sys

sys.path.insert(0, "/opt/trn_rl_repo")

import numpy as np
from ml_dtypes import bfloat16

import concourse.bass as bass
import concourse.bacc as bacc
import concourse.tile as tile
from concourse import mybir
from concourse import bass_utils

F32 = mybir.dt.float32
BF16 = mybir.dt.bfloat16
AF = mybir.ActivationFunctionType
ALU = mybir.AluOpType
AX = mybir.AxisListType

D = 1024
L = 128
NOBJ = 64
T = 16
ADIM = 8
NACT = 8
NCORES = 8
PAIRS = NOBJ * (NOBJ - 1)  # 4032
PPC = PAIRS // NCORES  # 504

LAST_EXEC_NS = None


# ---------------- host layout helpers ----------------

def _lhsT_tiles(W, n_m, n_k):
    """W [n_m*128, n_k*128] -> [128, n_m*n_k*128]; tile (m,k) at cols
    (m*n_k+k)*128, element [kk,p] = W[m*128+p, k*128+kk]."""
    return np.ascontiguousarray(
        W.reshape(n_m, 128, n_k, 128).transpose(3, 0, 2, 1).reshape(128, n_m * n_k * 128)
    )


def _gate_perm_rows(W):
    """Reorder rows of [4096, K] from (g,m,p) to block order c=m*4+g."""
    K = W.shape[1]
    return W.reshape(4, 8, 128, K).transpose(1, 0, 2, 3).reshape(32 * 128, K)


def _l2_tiles(W, n_k):
    """W [J, n_k*128] -> [128, n_k*J]; tile k at cols k*J, [kk,j]=W[j,k*128+kk]."""
    J = W.shape[0]
    return np.ascontiguousarray(W.reshape(J, n_k, 128).transpose(2, 1, 0).reshape(128, n_k * J))


def _vec_pm(v):
    """[n_m*128] -> [128, n_m] (p, m)."""
    return np.ascontiguousarray(v.reshape(-1, 128).T)


def _bf(a):
    return np.ascontiguousarray(a.astype(bfloat16))


def _f32(a):
    return np.ascontiguousarray(a.astype(np.float32))


# ---------------- device program ----------------

def _build(inputs_meta):
    """inputs_meta: dict name -> (shape, mybir dtype). Returns (nc, in_aps, out_aps)."""
    nc = bacc.Bacc(
        "TRN2", target_bir_lowering=False, debug=False,
        enable_asserts=False, num_devices=NCORES,
    )
    ins = {}
    for name, (shape, dt) in inputs_meta.items():
        ins[name] = nc.dram_tensor(name, list(shape), dt, kind="ExternalInput").ap()

    outs = {}
    for name, shape in [
        ("out_pol", (ADIM, 1)), ("out_val", (1, 1)), ("out_cf", (ADIM, 1)),
        ("out_cm", (NOBJ, NOBJ)), ("out_mechT", (3, PPC)),
        ("out_instr", (128, 8)), ("out_parse", (4, 1)), ("out_align", (ADIM, 1)),
        ("out_gl", (1, 1)), ("out_tl", (1, 1)), ("out_dbg", (128, 16)),
    ]:
        outs[name] = nc.dram_tensor(name, list(shape), F32, kind="ExternalOutput").ap()

    with tile.TileContext(nc) as tc:
        _emit(tc, nc, ins, outs)
    nc.compile()
    return nc


def _emit(tc, nc, ins, outs):
    import contextlib
    ctx = contextlib.ExitStack()
    with ctx:
        persist = ctx.enter_context(tc.tile_pool(name="persist", bufs=1))
        wbig = ctx.enter_context(tc.tile_pool(name="wbig", bufs=1))
        work = ctx.enter_context(tc.tile_pool(name="work", bufs=2))
        ps_big = ctx.enter_context(tc.tile_pool(name="ps_big", bufs=2, space="PSUM"))
        ps_xg = ctx.enter_context(tc.tile_pool(name="ps_xg", bufs=2, space="PSUM"))
        ps_sm = ctx.enter_context(tc.tile_pool(name="ps_sm", bufs=4, space="PSUM"))
        dram = ctx.enter_context(tc.tile_pool(name="scr", bufs=1, space="DRAM"))

        def sb(shape, dt, tag, pool=persist):
            t = pool.tile(list(shape), dt, tag=tag)
            return t

        def dma(dst_ap, src_ap):
            nc.sync.dma_start(dst_ap, src_ap)

        # ---- persistent SBUF tensors ----
        whh = sb((128, 32 * 8 * 128), BF16, "whh")          # 64KB/p resident
        xg = sb((128, L * 32), F32, "xg")                   # 16KB/p
        weT = sb((128, 8 * 128), BF16, "weT")
        bias_g = sb((128, 32), F32, "bias_g")
        state_bf = sb((128, 32), BF16, "state_bf")
        b_enc = sb((128, 8), F32, "b_enc")
        prev_sb = sb((128, 480), F32, "prev")
        curr_sb = sb((128, 480), F32, "curr")
        eye_sb = sb((NOBJ, NOBJ), F32, "eye")
        cmin_sb = sb((NOBJ, NOBJ), F32, "cmin")
        ident = sb((128, 128), F32, "ident")
        ones = sb((128, 1), F32, "ones")
        ae_f = sb((128, 64), F32, "ae_f")
        mechT = sb((128, 16 * PPC), BF16, "mechT")          # 16KB/p
        b_m1 = sb((128, 8), F32, "b_m1")
        wm2 = sb((128, 8 * 3), BF16, "wm2")
        b_m2 = sb((3, 1), F32, "b_m2")
        b_p1 = sb((128, 8), F32, "b_p1")
        wp2 = sb((128, 8 * 4), BF16, "wp2")
        b_p2 = sb((4, 1), F32, "b_p2")
        b_a1 = sb((128, 8), F32, "b_a1")
        wa2 = sb((128, 8), BF16, "wa2")
        bh1 = {k: sb((128, 8), F32, f"bh1_{k}") for k in ("pol", "val", "cf")}
        w2 = {
            "pol": sb((128, 8 * ADIM), BF16, "w2pol"),
            "val": sb((128, 8 * 1), BF16, "w2val"),
            "cf": sb((128, 8 * ADIM), BF16, "w2cf"),
        }
        b2 = {
            "pol": sb((ADIM, 1), F32, "b2pol"),
            "val": sb((1, 1), F32, "b2val"),
            "cf": sb((ADIM, 1), F32, "b2cf"),
        }

        # LSTM state
        h_bf = sb((128, 8), BF16, "h_bf")
        c_st = sb((128, 8), F32, "c_st")
        instr_acc = sb((128, 8), F32, "instr_acc")
        gates_sb = sb((128, 32), F32, "gates_sb")
        ig = sb((128, 8), F32, "ig")
        fg = sb((128, 8), F32, "fg")
        gg = sb((128, 8), F32, "gg")
        og = sb((128, 8), F32, "og")
        tch = sb((128, 8), F32, "tch")
        tmp8 = sb((128, 8), F32, "tmp8")
        h_fp = sb((128, 8), F32, "h_fp")

        # ---- input DMAs (small + resident first) ----
        dma(whh[:], ins["whh_t"])
        dma(weT[:], ins["weT"])
        dma(bias_g[:], ins["bias_g"])
        dma(state_bf[:], ins["state_bf"])
        dma(b_enc[:], ins["b_enc_pm"])
        dma(prev_sb[:], ins["prev"])
        dma(curr_sb[:], ins["curr"])
        dma(eye_sb[:], ins["eye"])
        dma(cmin_sb[:], ins["causal_matrix_in"])
        dma(ident[:], ins["ident"])
        dma(ones[:], ins["ones"])
        dma(ae_f[:], ins["aeT_f"])
        dma(mechT[:], ins["mech_inT"])
        dma(b_m1[:], ins["b_m1_pm"])
        dma(wm2[:], ins["wm2_t"])
        dma(b_m2[:], ins["b_m2_c"])
        dma(b_p1[:], ins["b_p1_pm"])
        dma(wp2[:], ins["wp2_t"])
        dma(b_p2[:], ins["b_p2_c"])
        dma(b_a1[:], ins["b_a1_pm"])
        dma(wa2[:], ins["wa2_t"])
        for k in ("pol", "val", "cf"):
            dma(bh1[k][:], ins[f"b_{k}1_pm"])
            dma(w2[k][:], ins[f"w{k}2_t"])
            dma(b2[k][:], ins[f"b_{k}2_c"])

        # ============ pre-loop compute ============

        # -- mech MLP (sharded over pairs) --
        wm1 = wbig.tile([128, 8 * 16 * 128], BF16, tag="big")
        dma(wm1[:], ins["wm1_t"])
        h1m = persist.tile([128, 8 * PPC], BF16, tag="h1m")
        for m in range(8):
            ps = ps_big.tile([128, PPC], F32, tag="big504")
            for k in range(16):
                nc.tensor.matmul(
                    ps[:], wm1[:, (m * 16 + k) * 128:(m * 16 + k + 1) * 128],
                    mechT[:, k * PPC:(k + 1) * PPC],
                    start=(k == 0), stop=(k == 15),
                )
            h1f = work.tile([128, PPC], F32, tag="h1f")
            nc.vector.tensor_tensor(h1f[:], ps[:], b_m1[:, m:m + 1].broadcast_to([128, PPC]), op=ALU.add)
            nc.scalar.activation(h1m[:, m * PPC:(m + 1) * PPC], h1f[:], AF.Relu)
        ps2 = ps_big.tile([3, PPC], F32, tag="big504")
        for k in range(8):
            nc.tensor.matmul(ps2[:], wm2[:, k * 3:(k + 1) * 3],
                             h1m[:, k * PPC:(k + 1) * PPC],
                             start=(k == 0), stop=(k == 7))
        mech_o = work.tile([3, PPC], F32, tag="mech_o")
        nc.vector.tensor_tensor(mech_o[:], ps2[:], b_m2[:, 0:1].broadcast_to([3, PPC]), op=ALU.add)
        dma(outs["out_mechT"], mech_o[:])

        # -- xg = we @ W_ih.T + bias (layout [p, t*32+c]) --
        wih = wbig.tile([128, 32 * 8 * 128], BF16, tag="big")
        dma(wih[:], ins["wih_t"])
        xg_v = xg[:].rearrange("p (t c) -> p c t", c=32)
        for c in range(32):
            psx = ps_xg.tile([128, 128], F32, tag="xg")
            for k in range(8):
                nc.tensor.matmul(psx[:], wih[:, (c * 8 + k) * 128:(c * 8 + k + 1) * 128],
                                 weT[:, k * 128:(k + 1) * 128],
                                 start=(k == 0), stop=(k == 7))
            nc.vector.tensor_tensor(xg_v[:, c], psx[:], bias_g[:, c:c + 1].broadcast_to([128, 128]), op=ALU.add)

        # -- state encoder GEMV --
        wenc = wbig.tile([128, 8 * 32 * 128], BF16, tag="big")
        dma(wenc[:], ins["wenc_t"])
        sps = ps_sm.tile([128, 8], F32, tag="small")
        for m in range(8):
            for k in range(32):
                nc.tensor.matmul(sps[:, m:m + 1],
                                 wenc[:, (m * 32 + k) * 128:(m * 32 + k + 1) * 128],
                                 state_bf[:, k:k + 1], start=(k == 0), stop=(k == 31))
        state_repr = persist.tile([128, 8], F32, tag="state_repr")
        nc.vector.tensor_tensor(state_repr[:], sps[:], b_enc[:], op=ALU.add)

        # -- causal graph stats --
        diff = work.tile([128, 480], F32, tag="diff")
        nc.vector.tensor_tensor(diff[:], curr_sb[:], prev_sb[:], op=ALU.not_equal)
        presP = persist.tile([128, 1024], F32, tag="presP")
        nc.vector.memset(presP[:], 0.0)
        eq = work.tile([128, 480], F32, tag="eq")
        prev3 = prev_sb[:].rearrange("p (t c) -> p t c", c=32)
        for o in range(NOBJ):
            nc.vector.tensor_scalar(eq[:], prev_sb[:], float(o), None, op0=ALU.is_equal)
            nc.vector.tensor_reduce(presP[:, o * 16:o * 16 + 15],
                                    eq[:].rearrange("p (t c) -> p t c", c=32),
                                    axis=AX.X, op=ALU.max)
        pres8 = persist.tile([128, 8], F32, tag="pres8")
        for j in range(8):
            tp = ps_xg.tile([128, 128], F32, tag="xg")
            nc.tensor.transpose(tp[:], presP[:, j * 128:(j + 1) * 128], ident[:])
            nc.vector.tensor_reduce(pres8[:, j:j + 1], tp[:], axis=AX.X, op=ALU.max)
        scrA = dram.tile([128, 8], F32, tag="scrA")
        dma(scrA[:], pres8[:])
        pres_lhsT = persist.tile([16, 64], F32, tag="pres_lhsT")
        dma(pres_lhsT[:].rearrange("t (j a) -> t j a", a=8),
            scrA[:].rearrange("(a t) j -> t j a", a=8, t=16))

        chs = work.tile([128, 64], F32, tag="chs")
        nc.vector.memset(chs[:], 0.0)
        nc.vector.tensor_reduce(chs[:, 0:15], diff[:].rearrange("p (t c) -> p t c", c=32),
                                axis=AX.X, op=ALU.min)
        nc.vector.tensor_reduce(chs[:, 32:47], diff[:].rearrange("p (t c) -> p t c", c=32),
                                axis=AX.X, op=ALU.max)
        tchp = ps_xg.tile([64, 128], F32, tag="xg")
        nc.tensor.transpose(tchp[:], chs[:], ident[:])
        chg0r = work.tile([16, 1], F32, tag="chg0r")
        chg1 = work.tile([16, 1], F32, tag="chg1")
        nc.vector.tensor_reduce(chg0r[:], tchp[0:16, :], axis=AX.X, op=ALU.min)
        nc.vector.tensor_reduce(chg1[:], tchp[32:48, :], axis=AX.X, op=ALU.max)
        chg0 = work.tile([16, 1], F32, tag="chg0")
        nc.vector.tensor_scalar(chg0[:], chg0r[:], -1.0, 1.0, op0=ALU.mult, op1=ALU.add)
        changed = persist.tile([16, 64], F32, tag="changed")
        nc.vector.memset(changed[:], 0.0)
        nc.vector.tensor_copy(changed[:, 0:1], chg0[:])
        nc.vector.tensor_copy(changed[:, 1:2], chg1[:])

        dps = ps_sm.tile([64, 64], F32, tag="small")
        nc.tensor.matmul(dps[:], pres_lhsT[0:15, :], changed[0:15, :], start=True, stop=True)
        t1 = work.tile([64, 64], F32, tag="cmt")
        nc.vector.tensor_scalar(t1[:], dps[:], 0.01, None, op0=ALU.mult)
        offd = work.tile([64, 64], F32, tag="offd")
        nc.vector.tensor_scalar(offd[:], eye_sb[:], -1.0, 1.0, op0=ALU.mult, op1=ALU.add)
        nc.vector.tensor_tensor(t1[:], t1[:], offd[:], op=ALU.mult)
        nc.vector.tensor_tensor(t1[:], t1[:], cmin_sb[:], op=ALU.add)
        cm = persist.tile([64, 64], F32, tag="cm")
        nc.scalar.activation(cm[:], t1[:], AF.Sigmoid)
        dma(outs["out_cm"], cm[:])

        # causal_mat losses
        dfm = work.tile([64, 64], F32, tag="dfm")
        nc.vector.tensor_tensor(dfm[:], cm[:], eye_sb[:], op=ALU.subtract)
        sqr = work.tile([64, 1], F32, tag="sqr")
        absr = work.tile([64, 1], F32, tag="absr")
        nc.scalar.activation(dfm[:], dfm[:], AF.Square, accum_out=sqr[:])
        cmabs = work.tile([64, 64], F32, tag="cmabs")
        nc.scalar.activation(cmabs[:], cm[:], AF.Abs, accum_out=absr[:])
        sq_ps = ps_sm.tile([1, 1], F32, tag="small")
        nc.tensor.matmul(sq_ps[:], sqr[:], ones[0:64, :], start=True, stop=True)
        ab_ps = ps_sm.tile([1, 1], F32, tag="small")
        nc.tensor.matmul(ab_ps[:], absr[:], ones[0:64, :], start=True, stop=True)
        sq_s = persist.tile([1, 1], F32, tag="sq_s")
        ab_s = persist.tile([1, 1], F32, tag="ab_s")
        nc.vector.tensor_scalar(sq_s[:], sq_ps[:], 1.0 / 4096.0, None, op0=ALU.mult)
        nc.vector.tensor_scalar(ab_s[:], ab_ps[:], 0.1 / 4096.0, None, op0=ALU.mult)

        # -- causal_repr GEMV (W_rand) --
        scrB = dram.tile([64, 64], F32, tag="scrB")
        dma(scrB[:], cm[:])
        cfl = work.tile([128, 32], F32, tag="cfl")
        dma(cfl[:], scrB[:].rearrange("(c a) j -> (a j) c", c=32, a=2))
        cfl_bf = work.tile([128, 32], BF16, tag="cfl_bf")
        nc.vector.tensor_copy(cfl_bf[:], cfl[:])
        wrand = wbig.tile([128, 8 * 32 * 128], BF16, tag="big")
        dma(wrand[:], ins["wrand_t"])
        cps = ps_sm.tile([128, 8], F32, tag="small")
        for m in range(8):
            for k in range(32):
                nc.tensor.matmul(cps[:, m:m + 1],
                                 wrand[:, (m * 32 + k) * 128:(m * 32 + k + 1) * 128],
                                 cfl_bf[:, k:k + 1], start=(k == 0), stop=(k == 31))
        causal_repr = persist.tile([128, 8], F32, tag="causal_repr")
        nc.vector.tensor_copy(causal_repr[:], cps[:])
        dbg = persist.tile([128, 16], F32, tag="dbg")
        nc.vector.tensor_copy(dbg[:, 0:8], state_repr[:])
        nc.vector.tensor_copy(dbg[:, 8:16], causal_repr[:])
        dma(outs["out_dbg"], dbg[:])

        # ============ LSTM loop ============
        nc.vector.memset(h_bf[:], 0.0)
        nc.vector.memset(c_st[:], 0.0)
        nc.vector.memset(instr_acc[:], 0.0)
        gates_ps = ps_sm.tile([128, 32], F32, tag="small")
        gv = gates_sb[:].rearrange("p (m g) -> p g m", g=4)

        with tc.For_i(0, L * 32, 32, hint_engines=(mybir.EngineType.PE,)) as iv:
            for c in range(32):
                for k in range(8):
                    nc.tensor.matmul(
                        gates_ps[:, c:c + 1],
                        whh[:, (c * 8 + k) * 128:(c * 8 + k + 1) * 128],
                        h_bf[:, k:k + 1], start=(k == 0), stop=(k == 7),
                    )
            nc.vector.tensor_tensor(gates_sb[:], gates_ps[:],
                                    xg[:, bass.ds(iv, 32)], op=ALU.add)
            nc.scalar.activation(ig[:], gv[:, 0], AF.Sigmoid)
            nc.scalar.activation(fg[:], gv[:, 1], AF.Sigmoid)
            nc.scalar.activation(gg[:], gv[:, 2], AF.Tanh)
            nc.scalar.activation(og[:], gv[:, 3], AF.Sigmoid)
            nc.vector.tensor_tensor(tmp8[:], ig[:], gg[:], op=ALU.mult)
            nc.vector.tensor_tensor(c_st[:], fg[:], c_st[:], op=ALU.mult)
            nc.vector.tensor_tensor(c_st[:], c_st[:], tmp8[:], op=ALU.add)
            nc.scalar.activation(tch[:], c_st[:], AF.Tanh)
            nc.vector.tensor_tensor(h_fp[:], og[:], tch[:], op=ALU.mult)
            nc.vector.tensor_copy(h_bf[:], h_fp[:])
            nc.vector.tensor_tensor(instr_acc[:], instr_acc[:], h_fp[:], op=ALU.add)

        # ============ post-loop ============
        instr_sb = persist.tile([128, 8], F32, tag="instr_sb")
        nc.vector.tensor_scalar(instr_sb[:], instr_acc[:], 1.0 / float(L), None, op0=ALU.mult)
        dma(outs["out_instr"], instr_sb[:])

        comb = persist.tile([128, 24], BF16, tag="comb")
        nc.vector.tensor_copy(comb[:, 0:8], state_repr[:])
        nc.vector.tensor_copy(comb[:, 8:16], causal_repr[:])
        nc.vector.tensor_copy(comb[:, 16:24], instr_sb[:])

        # -- three heads --
        for key, wname in (("pol", "wpol1_t"), ("val", "wval1_t"), ("cf", "wcf1_t")):
            wh = wbig.tile([128, 8 * 24 * 128], BF16, tag="big")
            dma(wh[:], ins[wname])
            hps = ps_sm.tile([128, 8], F32, tag="small")
            for m in range(8):
                for k in range(24):
                    nc.tensor.matmul(hps[:, m:m + 1],
                                     wh[:, (m * 24 + k) * 128:(m * 24 + k + 1) * 128],
                                     comb[:, k:k + 1], start=(k == 0), stop=(k == 23))
            h1 = work.tile([128, 8], F32, tag="h1")
            nc.vector.tensor_tensor(h1[:], hps[:], bh1[key][:], op=ALU.add)
            h1b = work.tile([128, 8], BF16, tag="h1b")
            nc.scalar.activation(h1b[:], h1[:], AF.Relu)
            J = 1 if key == "val" else ADIM
            l2 = ps_sm.tile([J, 1], F32, tag="small")
            for k in range(8):
                nc.tensor.matmul(l2[:], w2[key][:, k * J:(k + 1) * J],
                                 h1b[:, k:k + 1], start=(k == 0), stop=(k == 7))
            res = work.tile([J, 1], F32, tag=f"res_{key}")
            nc.vector.tensor_tensor(res[:], l2[:], b2[key][0:J, 0:1], op=ALU.add)
            dma(outs[f"out_{key}"], res[:])

        # -- causal_parse --
        wp1 = wbig.tile([128, 8 * 8 * 128], BF16, tag="big")
        dma(wp1[:], ins["wp1_t"])
        pps = ps_sm.tile([128, 8], F32, tag="small")
        for m in range(8):
            for k in range(8):
                nc.tensor.matmul(pps[:, m:m + 1],
                                 wp1[:, (m * 8 + k) * 128:(m * 8 + k + 1) * 128],
                                 comb[:, 16 + k:17 + k], start=(k == 0), stop=(k == 7))
        ph1 = work.tile([128, 8], F32, tag="ph1")
        nc.vector.tensor_tensor(ph1[:], pps[:], b_p1[:], op=ALU.add)
        ph1b = work.tile([128, 8], BF16, tag="ph1b")
        nc.scalar.activation(ph1b[:], ph1[:], AF.Relu)
        pl2 = ps_sm.tile([4, 1], F32, tag="small")
        for k in range(8):
            nc.tensor.matmul(pl2[:], wp2[:, k * 4:(k + 1) * 4], ph1b[:, k:k + 1],
                             start=(k == 0), stop=(k == 7))
        parse = work.tile([4, 1], F32, tag="parse")
        nc.vector.tensor_tensor(parse[:], pl2[:], b_p2[0:4, 0:1], op=ALU.add)
        dma(outs["out_parse"], parse[:])

        # -- alignment --
        ibc = persist.tile([128, 64], BF16, tag="ibc")
        for k in range(8):
            nc.vector.tensor_copy(ibc[:, k * 8:(k + 1) * 8],
                                  comb[:, 16 + k:17 + k].broadcast_to([128, 8]))
        ae_bf = persist.tile([128, 64], BF16, tag="ae_bf")
        nc.vector.tensor_copy(ae_bf[:], ae_f[:])
        wa1 = wbig.tile([128, 8 * 16 * 128], BF16, tag="big")
        dma(wa1[:], ins["wa1_t"])
        ha1 = persist.tile([128, 64], BF16, tag="ha1")
        for m in range(8):
            aps = ps_sm.tile([128, 8], F32, tag="small")
            for k in range(16):
                rhs = ibc[:, k * 8:(k + 1) * 8] if k < 8 else ae_bf[:, (k - 8) * 8:(k - 7) * 8]
                nc.tensor.matmul(aps[:],
                                 wa1[:, (m * 16 + k) * 128:(m * 16 + k + 1) * 128],
                                 rhs, start=(k == 0), stop=(k == 15))
            ah = work.tile([128, 8], F32, tag="ah")
            nc.vector.tensor_tensor(ah[:], aps[:], b_a1[:, m:m + 1].broadcast_to([128, 8]), op=ALU.add)
            nc.scalar.activation(ha1[:, m * 8:(m + 1) * 8], ah[:], AF.Relu)
        al_ps = ps_sm.tile([ADIM, 1], F32, tag="small")
        for m in range(8):
            nc.tensor.matmul(al_ps[:], ha1[:, m * 8:(m + 1) * 8], wa2[:, m:m + 1],
                             start=(m == 0), stop=(m == 7))
        align = work.tile([ADIM, 1], F32, tag="align")
        nc.vector.tensor_scalar(align[:], al_ps[:], float(B_A2_VAL[0]), None, op0=ALU.add)
        dma(outs["out_align"], align[:])

        # -- grounding loss + total loss --
        ir = work.tile([128, 1], F32, tag="ir")
        nc.vector.tensor_reduce(ir[:], instr_sb[:], axis=AX.X, op=ALU.add)
        mi_ps = ps_sm.tile([1, 1], F32, tag="small")
        nc.tensor.matmul(mi_ps[:], ir[:], ones[:], start=True, stop=True)
        ar = work.tile([128, 1], F32, tag="ar")
        nc.vector.tensor_reduce(ar[:], ae_f[:], axis=AX.X, op=ALU.add)
        ma_ps = ps_sm.tile([1, 1], F32, tag="small")
        nc.tensor.matmul(ma_ps[:], ar[:], ones[:], start=True, stop=True)
        mi_s = work.tile([1, 1], F32, tag="mi_s")
        ma_s = work.tile([1, 1], F32, tag="ma_s")
        nc.vector.tensor_scalar(mi_s[:], mi_ps[:], 1.0 / 1024.0, None, op0=ALU.mult)
        nc.vector.tensor_scalar(ma_s[:], ma_ps[:], 1.0 / 8192.0, None, op0=ALU.mult)
        gl = work.tile([1, 1], F32, tag="gl")
        nc.vector.tensor_tensor(gl[:], mi_s[:], ma_s[:], op=ALU.subtract)
        nc.scalar.activation(gl[:], gl[:], AF.Square)
        dma(outs["out_gl"], gl[:])
        tl = work.tile([1, 1], F32, tag="tl")
        nc.vector.tensor_tensor(tl[:], sq_s[:], gl[:], op=ALU.add)
        nc.vector.tensor_tensor(tl[:], tl[:], ab_s[:], op=ALU.add)
        dma(outs["out_tl"], tl[:])


B_A2_VAL = np.zeros(1, np.float32)  # set per-call before build


# ---------------- entry point ----------------

def kernel(**inputs):
    global LAST_EXEC_NS, B_A2_VAL
    g = {k: np.asarray(v) for k, v in inputs.items()}

    # host gathers / layout staging (index + layout only)
    we = g["word_emb"][g["instruction_tokens"][0].astype(np.int64)]      # [128, 1024]
    ae = g["act_emb"][g["actions"].astype(np.int64)]                     # [8, 1024]
    sh = g["state_history"][0].reshape(T, -1).astype(np.float32)         # [16, 4096]
    prev, curr = sh[:-1], sh[1:]                                         # [15, 4096]

    ii, jj = np.meshgrid(np.arange(NOBJ), np.arange(NOBJ), indexing="ij")
    m = ii != jj
    o1, o2 = ii[m], jj[m]
    B_A2_VAL = _f32(g["b_a2"]).reshape(1)

    common = {
        "whh_t": _bf(_lhsT_tiles(_gate_perm_rows(g["W_hh"]), 32, 8)),
        "wih_t": _bf(_lhsT_tiles(_gate_perm_rows(g["W_ih"]), 32, 8)),
        "weT": _bf(we.reshape(128, 8, 128).transpose(2, 1, 0).reshape(128, 1024)),
        "bias_g": _f32((g["b_ih"] + g["b_hh"]).reshape(4, 8, 128).transpose(1, 0, 2)
                       .reshape(32, 128).T),
        "state_bf": _bf(g["state"].reshape(-1).reshape(32, 128).T),
        "b_enc_pm": _f32(_vec_pm(g["b_enc"])),
        "wenc_t": _bf(_lhsT_tiles(g["W_enc"], 8, 32)),
        "wrand_t": _bf(_lhsT_tiles(g["W_rand"], 8, 32)),
        "prev": _f32(prev.reshape(15, 32, 128).transpose(2, 0, 1).reshape(128, 480)),
        "curr": _f32(curr.reshape(15, 32, 128).transpose(2, 0, 1).reshape(128, 480)),
        "eye": _f32(np.eye(NOBJ)),
        "causal_matrix_in": _f32(g["causal_matrix"]),
        "ident": _f32(np.eye(128)),
        "ones": _f32(np.ones((128, 1))),
        "aeT_f": _f32(ae.reshape(8, 8, 128).transpose(2, 1, 0).reshape(128, 64)),
        "wm1_t": _bf(_lhsT_tiles(g["W_m1"], 8, 16)),
        "b_m1_pm": _f32(_vec_pm(g["b_m1"])),
        "wm2_t": _bf(_l2_tiles(g["W_m2"], 8)),
        "b_m2_c": _f32(g["b_m2"].reshape(3, 1)),
        "wp1_t": _bf(_lhsT_tiles(g["W_p1"], 8, 8)),
        "b_p1_pm": _f32(_vec_pm(g["b_p1"])),
        "wp2_t": _bf(_l2_tiles(g["W_p2"], 8)),
        "b_p2_c": _f32(g["b_p2"].reshape(4, 1)),
        "wa1_t": _bf(_lhsT_tiles(g["W_a1"], 8, 16)),
        "b_a1_pm": _f32(_vec_pm(g["b_a1"])),
        "wa2_t": _bf(_l2_tiles(g["W_a2"], 8)),
        "wpol1_t": _bf(_lhsT_tiles(g["W_pol1"], 8, 24)),
        "b_pol1_pm": _f32(_vec_pm(g["b_pol1"])),
        "wpol2_t": _bf(_l2_tiles(g["W_pol2"], 8)),
        "b_pol2_c": _f32(g["b_pol2"].reshape(ADIM, 1)),
        "wval1_t": _bf(_lhsT_tiles(g["W_val1"], 8, 24)),
        "b_val1_pm": _f32(_vec_pm(g["b_val1"])),
        "wval2_t": _bf(_l2_tiles(g["W_val2"], 8)),
        "b_val2_c": _f32(g["b_val2"].reshape(1, 1)),
        "wcf1_t": _bf(_lhsT_tiles(g["W_cf1"], 8, 24)),
        "b_cf1_pm": _f32(_vec_pm(g["b_cf1"])),
        "wcf2_t": _bf(_l2_tiles(g["W_cf2"], 8)),
        "b_cf2_c": _f32(g["b_cf2"].reshape(ADIM, 1)),
    }

    in_maps = []
    for core in range(NCORES):
        s = slice(core * PPC, (core + 1) * PPC)
        mech_in = np.concatenate([g["obj_emb"][o1[s]], g["obj_emb"][o2[s]]], axis=1)  # [504, 2048]
        mechT = mech_in.T.reshape(16, 128, PPC).transpose(1, 0, 2).reshape(128, 16 * PPC)
        in_maps.append({**common, "mech_inT": _bf(mechT)})

    meta = {k: (v.shape, BF16 if v.dtype == bfloat16 else F32) for k, v in in_maps[0].items()}
    nc = _build(meta)

    trace = bool(os.environ.get("KERNEL_TRACE"))
    res = bass_utils.run_bass_kernel_spmd(nc, in_maps, core_ids=list(range(NCORES)),
                                          trace=trace)
    LAST_EXEC_NS = res.exec_time_ns
    r0 = res.results[0]

    policy = r0["out_pol"][:, 0][None, :].astype(np.float32)
    value = r0["out_val"].reshape(1, 1).astype(np.float32)
    cf = r0["out_cf"][:, 0][None, :].astype(np.float32)
    cm = r0["out_cm"].astype(np.float32)
    mech = np.concatenate([res.results[c]["out_mechT"].T for c in range(NCORES)], axis=0)
    causal_pred = np.tile(mech, (T - 1, 1))[:, None, :].astype(np.float32)
    instr = r0["out_instr"].T.reshape(1, D).astype(np.float32)
    parse = r0["out_parse"][:, 0][None, :].astype(np.float32)
    align = r0["out_align"][:, 0][:, None, None].astype(np.float32)
    gl = np.float32(r0["out_gl"][0, 0])
    tl = np.float32(r0["out_tl"][0, 0])
    return (policy, value, cf, cm, causal_pred, instr, parse, align, gl, tl)
